# revision 66
# baseline (speedup 1.0000x reference)
"""GCNConv (DGL GraphConv norm='both') on 8 Trainium2 NeuronCores.

out = D_dst^-1/2 * A * (D_src^-1/2 * X * W) + b
  X: [100000, 32] f32, edge_index: [2, 1600000] (src, dst), W: [32, 32], b: [32]

Sharding: nodes are range-partitioned across the 8 cores (12500 each); each
core owns the aggregation for dst nodes in its range (graph/data parallel).
The host only buckets/sorts/remaps integer indices and re-lays-out tensors;
all floating-point math runs on device.

Device algorithm (single SPMD program, NO collectives):
  Phase 1 (replicated): every core computes the full message table
           m = (x @ W) * outdeg^-1/2 for ALL nodes from a host-transposed
           xT [32, N] input (lhsT for the PE without any on-device
           transposes), writing f16 rows packed into a 256B-strided DRAM
           table (the dma_gather element granularity).
  Phase 2: edges are pre-sorted by dst. Each 128-dst window's edges are split
           by src quarter (4 groups) and padded per (window, group) to a
           common block count RQ (SPMD-uniform). Messages m[src] are fetched
           with dma_gather (elem = 256B row, per-quarter table slice so the
           int16 index fits). Per window, a one-hot matrix (edge -> local
           dst) is built in one vector-engine is_equal op and the 4*RQ
           accumulating matmuls produce the window's aggregation directly in
           PSUM. Scale by indeg^-1/2, add bias.
"""

import os
import sys

import numpy as np

for _p in ("/opt/trn_rl_repo", "/root/.axon_site/_ro/trn_rl_repo"):
    if os.path.isdir(_p) and _p not in sys.path:
        sys.path.insert(0, _p)

N_NODES = 100000
N_CORES = 8
NPC = N_NODES // N_CORES  # 12500 nodes per core
DIN = 32
DOUT = 32
P = 128  # partitions
MROW = 128  # f16 elements per padded m row (256 bytes)
NTILE = (NPC + P - 1) // P  # 98 dst windows per core
NPAD = NTILE * P
NG = 4  # src quarters
QN = N_NODES // NG  # 25000 nodes per quarter (int16-addressable)

QNP = 25088  # 128-aligned padded quarter rows (nodes g*QN .. g*QN+QNP)
NCH = NG * (QNP // P)  # 784 column chunks (quarter-padded layout)
NALL = 100096  # xT columns incl. pad

XCHUNK = 4096  # nodes per xT SBUF chunk (phase 1)
SUPER = 2048  # nodes per m-table write batch


SPAN_W = 6  # windows per gather span / overflow group


def _stream_layout(RQW, OVN):
    """Shared host/device slot-stream geometry.

    RQW[w][g]: main 128-edge blocks for (window, quarter), capped at 4.
    OVN[sp][g]: overflow blocks for (span, quarter) — edges beyond the main
    cap of any window in the span; their one-hot is built against a
    span-relative class id (cls + 128*(w - sp*SPAN_W)).

    Quarter-g gather stream is span-major: [mains w0..w5][overflow].
    Returns dicts of offsets.
    """
    nspan = (NTILE + SPAN_W - 1) // SPAN_W
    # quarter stream: soff[g][sp] = slot offset of span sp's block
    soff = [[0] for _ in range(NG)]
    moff = [dict() for _ in range(NG)]  # (g, w) -> main offset in stream
    for g in range(NG):
        for sp in range(nspan):
            off = soff[g][sp]
            for w in range(sp * SPAN_W, min((sp + 1) * SPAN_W, NTILE)):
                moff[g][w] = off
                off += RQW[w][g]
            off += OVN[sp][g]
            soff[g].append(off)
    GSL = [soff[g][nspan] for g in range(NG)]
    # window-major one-hot slot layout: mains (w-major, g-major within w),
    # then per-span overflow blocks appended after all mains
    WSLW = [sum(RQW[w]) for w in range(NTILE)]
    woff = [0]
    for w in range(NTILE):
        woff.append(woff[-1] + WSLW[w])
    ov_woff = [woff[-1]]  # overflow slot offset per (span, g), sp-major
    ovoff = [dict() for _ in range(NG)]
    off = woff[-1]
    for sp in range(nspan):
        for g in range(NG):
            ovoff[g][sp] = off
            off += OVN[sp][g]
    SLOTS = off
    return dict(nspan=nspan, soff=soff, moff=moff, GSL=GSL, WSLW=WSLW,
                woff=woff, ovoff=ovoff, SLOTS=SLOTS)


def _build_program(RQW, OVN):
    """Build the SPMD program (see _stream_layout for the slot geometry)."""
    from concourse import bacc, bass, mybir, tile

    f32 = mybir.dt.float32
    f16 = mybir.dt.float16
    i16 = mybir.dt.int16
    i32 = mybir.dt.int32
    Alu = mybir.AluOpType
    Act = mybir.ActivationFunctionType

    lay = _stream_layout(RQW, OVN)
    nspan = lay["nspan"]
    soff = lay["soff"]
    moff = lay["moff"]
    GSL = lay["GSL"]
    WSLW = lay["WSLW"]
    woff = lay["woff"]
    ovoff = lay["ovoff"]
    SLOTS = lay["SLOTS"]
    WSLMAX = max(WSLW)
    OVW = SPAN_W * P  # overflow one-hot width (span-relative classes)
    OVMAX = max(1, max(sum(OVN[sp]) for sp in range(nspan)))
    max_span_sl = max(
        soff[g][sp + 1] - soff[g][sp]
        for g in range(NG) for sp in range(nspan)
    )

    nc = bacc.Bacc(
        "TRN2",
        target_bir_lowering=False,
        debug=False,
        enable_asserts=False,
        num_devices=N_CORES,
    )

    # ---- I/O ----
    xT = nc.dram_tensor("xT", [DIN, NALL], f32, kind="ExternalInput")
    w_in = nc.dram_tensor("w_in", [DIN, DOUT], f32, kind="ExternalInput")
    b_rep = nc.dram_tensor("b_rep", [P, DOUT], f32, kind="ExternalInput")
    # per-quarter gather indices, stream-ordered (window-major runs)
    qidx = [
        nc.dram_tensor(f"qidx{g}", [P, GSL[g] * 8], i16, kind="ExternalInput")
        for g in range(NG)
    ]
    dstloc = nc.dram_tensor("dstloc", [P, SLOTS], f16, kind="ExternalInput")
    iota_in = nc.dram_tensor("iota_in", [P, OVW], f16, kind="ExternalInput")
    ocnt_in = nc.dram_tensor("ocnt", [P, NCH], i32, kind="ExternalInput")
    icnt_in = nc.dram_tensor("icnt", [P, NTILE], i32, kind="ExternalInput")
    out_d = nc.dram_tensor("out_d", [NPAD, DOUT], f32, kind="ExternalOutput")

    # ---- internal DRAM: per-quarter message tables, 256B-strided rows ----
    # Separate tensors give the dependency tracker precise quarter-level
    # edges, so quarter-g gathers start as soon as quarter g is written.
    m_q = [
        nc.dram_tensor(f"m_q{g}", [QNP, MROW], f16, kind="Internal")
        for g in range(NG)
    ]

    with tile.TileContext(nc) as tc:
        with (
            tc.tile_pool(name="const", bufs=1) as cpool,
            tc.tile_pool(name="xload", bufs=3) as xpool,
            tc.tile_pool(name="work", bufs=3) as wpool,
            tc.tile_pool(name="gath", bufs=2) as gpool,
            tc.tile_pool(name="psum", bufs=1, space="PSUM") as ppool,
            tc.tile_pool(name="psum2", bufs=2, space="PSUM") as ppool2,
        ):
            # ---- load constants ----
            w_t = cpool.tile([DIN, DOUT], f32)
            nc.sync.dma_start(out=w_t[:], in_=w_in[:])
            b_t = cpool.tile([P, DOUT], f32)
            nc.sync.dma_start(out=b_t[:], in_=b_rep[:])
            iota_t = cpool.tile([P, OVW], f16)
            nc.sync.dma_start(out=iota_t[:], in_=iota_in[:])
            qidx_t = []
            for g in range(NG):
                t = cpool.tile([P, GSL[g] * 8], i16, tag=f"qidx{g}")
                nc.sync.dma_start(out=t[:], in_=qidx[g][:])
                qidx_t.append(t)
            dst_t = cpool.tile([P, SLOTS], f16)
            nc.sync.dma_start(out=dst_t[:], in_=dstloc[:])
            ocnt_t = cpool.tile([P, NCH], i32)
            nc.sync.dma_start(out=ocnt_t[:], in_=ocnt_in[:])
            icnt_t = cpool.tile([P, NTILE], i32)
            nc.sync.dma_start(out=icnt_t[:], in_=icnt_in[:])

            # ---- out-degree norm for ALL nodes (node n = c*128 + p) ----
            ns_all = cpool.tile([P, NCH], f32)
            odeg = wpool.tile([P, NCH], f32, tag="odeg")
            nc.vector.tensor_scalar_max(out=odeg[:], in0=ocnt_t[:], scalar1=1.0)
            osq = wpool.tile([P, NCH], f32, tag="osq")
            nc.scalar.activation(out=osq[:], in_=odeg[:], func=Act.Sqrt)
            nc.vector.reciprocal(out=ns_all[:], in_=osq[:])

            # ---- in-degree norm (packed position l = w*128 + p) ----
            nd_all = cpool.tile([P, NTILE], f32)
            ideg = wpool.tile([P, NTILE], f32, tag="ideg")
            nc.vector.tensor_scalar_max(out=ideg[:], in0=icnt_t[:], scalar1=1.0)
            isq = wpool.tile([P, NTILE], f32, tag="isq")
            nc.scalar.activation(out=isq[:], in_=ideg[:], func=Act.Sqrt)
            nc.vector.reciprocal(out=nd_all[:], in_=isq[:])

            # ---- gather machinery (shared by phases 1 and 2) ----
            q_tiles = [[None] * NG for _ in range(nspan)]

            def ensure_q(sp, g):
                if q_tiles[sp][g] is not None:
                    return
                s0 = soff[g][sp]          # slot offset within group-g stream
                nsl = soff[g][sp + 1] - s0
                if nsl == 0:
                    q_tiles[sp][g] = False
                    return
                n_idx = nsl * P
                qt = gpool.tile([P, max_span_sl, MROW], f16, tag=f"q{g}",
                                name=f"qt{g}")
                nc.gpsimd.dma_gather(
                    out_ap=qt[:, :nsl, :],
                    in_ap=m_q[g][0:QN, :],
                    idxs_ap=qidx_t[g][:, s0 * 8:(s0 + nsl) * 8],
                    num_idxs=n_idx,
                    num_idxs_reg=n_idx,
                    elem_size=MROW,
                    single_packet=False,
                )
                q_tiles[sp][g] = qt

            def ensure_span(sp):
                for g in range(NG):
                    ensure_q(sp, g)

            def _early_gather(g):
                for sp in range(min(2, nspan)):
                    ensure_q(sp, g)

            # ---- phase 1 (replicated): m = (x @ W) * ns for ALL nodes ----
            # xT chunk -> 128-node matmuls -> psum groups of 8 -> DVE scale
            # to f16 -> packed 64B-row writes into the 256B-strided table.
            G4 = 8  # 128-node chunks per psum group
            for g in range(NG):
                for x0 in range(0, QNP, XCHUNK):
                    xn = min(XCHUNK, QNP - x0)
                    xc = xpool.tile([DIN, XCHUNK], f32, tag="xc")
                    nc.sync.dma_start(
                        out=xc[:, :xn],
                        in_=xT[:, g * QN + x0:g * QN + x0 + xn],
                    )
                    for s0 in range(0, xn, SUPER):
                        sn = min(SUPER, xn - s0)
                        ngrp = sn // P  # 128-node groups this super-chunk
                        m_sb = wpool.tile(
                            [P, SUPER // P, DOUT], f16, tag="m_sb"
                        )
                        for q0 in range(0, ngrp, G4):
                            qn = min(G4, ngrp - q0)
                            pt = ppool2.tile([P, G4, DOUT], f32)
                            for j in range(qn):
                                nc.tensor.matmul(
                                    out=pt[:, j, :],
                                    lhsT=xc[:, s0 + (q0 + j) * P:
                                            s0 + (q0 + j + 1) * P],
                                    rhs=w_t[:],
                                    start=True, stop=True,
                                )
                            c0 = (g * QNP + x0 + s0) // P + q0
                            nc.vector.tensor_tensor(
                                out=m_sb[:, q0:q0 + qn, :],
                                in0=pt[:, :qn, :],
                                in1=ns_all[:, c0:c0 + qn].unsqueeze(2)
                                .to_broadcast([P, qn, DOUT]),
                                op=Alu.mult,
                            )
                        n0 = x0 + s0
                        nc.sync.dma_start(
                            out=m_q[g][n0:n0 + sn, 0:DOUT].rearrange(
                                "(gg p) c -> p gg c", p=P
                            ),
                            in_=m_sb[:, :ngrp, :],
                        )
                # early-emit the first spans' quarter-g gathers right after
                # quarter g is written: their HWDGE-queue sem waits then
                # cover only the DMAs emitted so far, so they overlap the
                # rest of phase 1 instead of waiting for all of it
                _early_gather(g)

            # ---- phase 2: per-quarter gathers + windowed one-hot matmuls --
            for sp in range(nspan):
                ensure_span(sp)
                if sp + 1 < nspan:
                    ensure_span(sp + 1)  # prefetch next span
                w0 = sp * SPAN_W
                wlist = list(range(w0, min(w0 + SPAN_W, NTILE)))
                nw = len(wlist)
                novt = sum(OVN[sp])  # overflow slots this span (all quarters)
                # overflow one-hot first (span-relative classes), so each
                # window runs [ov matmuls, main matmuls, psum copy] and the
                # copy fires as soon as that window's own slots are done
                if novt:
                    ov0 = ovoff[0][sp]  # quarters' ov blocks are contiguous
                    ohv = wpool.tile([P, OVMAX, OVW], f16, tag="ohov")
                    nc.vector.tensor_tensor(
                        out=ohv[:, :novt, :],
                        in0=iota_t[:].unsqueeze(1).to_broadcast([P, novt, OVW]),
                        in1=dst_t[:, ov0:ov0 + novt]
                        .unsqueeze(2).to_broadcast([P, novt, OVW]),
                        op=Alu.is_equal,
                    )
                so = wpool.tile([P, SPAN_W, DOUT + 1], f32, tag="spanout")
                for t, w in enumerate(wlist):
                    wsl = WSLW[w]
                    total_k = wsl + novt
                    if total_k == 0:
                        nc.vector.memset(so[:, t:t + 1, 0:DOUT], 0.0)
                        continue
                    psw = ppool.tile([P, DOUT], f32, tag=f"ps{t}",
                                     name=f"ps{t}")
                    k = 0
                    s_local = 0
                    for g in range(NG):
                        qt = q_tiles[sp][g]
                        lo = soff[g][sp + 1] - soff[g][sp] - OVN[sp][g]
                        for s in range(OVN[sp][g]):
                            nc.tensor.matmul(
                                out=psw[:],
                                lhsT=ohv[:, s_local, t * P:(t + 1) * P],
                                rhs=qt[:, lo + s, 0:DOUT],
                                start=(k == 0),
                                stop=(k == total_k - 1),
                            )
                            k += 1
                            s_local += 1
                    oh = wpool.tile([P, WSLMAX, P + 1], f16, tag="onehot")
                    nc.vector.tensor_tensor(
                        out=oh[:, :wsl, 0:P],
                        in0=iota_t[:, 0:P].unsqueeze(1)
                        .to_broadcast([P, wsl, P]),
                        in1=dst_t[:, woff[w]:woff[w] + wsl]
                        .unsqueeze(2).to_broadcast([P, wsl, P]),
                        op=Alu.is_equal,
                    )
                    s_in_w = 0  # slot index within window (g-major)
                    for g in range(NG):
                        qt = q_tiles[sp][g]
                        lo = moff[g][w] - soff[g][sp]
                        for r in range(RQW[w][g]):
                            nc.tensor.matmul(
                                out=psw[:],
                                lhsT=oh[:, s_in_w, 0:P],
                                rhs=qt[:, lo + r, 0:DOUT],
                                start=(k == 0),
                                stop=(k == total_k - 1),
                            )
                            k += 1
                            s_in_w += 1
                    # psum -> SBUF with the indeg^-1/2 norm fused as the
                    # per-partition activation scale
                    nc.scalar.activation(
                        out=so[:, t:t + 1, 0:DOUT],
                        in_=psw[:].unsqueeze(1),
                        func=Act.Copy,
                        scale=nd_all[:, w:w + 1],
                    )
                # bias add + store this span's windows (node l = w*128 + p)
                nc.vector.tensor_tensor(
                    out=so[:, :nw, 0:DOUT], in0=so[:, :nw, 0:DOUT],
                    in1=b_t[:].unsqueeze(1).to_broadcast([P, nw, DOUT]),
                    op=Alu.add,
                )
                nc.sync.dma_start(
                    out=out_d[w0 * P:(w0 + nw) * P, :].rearrange(
                        "(w p) c -> p w c", p=P
                    ),
                    in_=so[:, :nw, 0:DOUT],
                )
                q_tiles[sp] = None  # allow pool slot reuse

    nc.compile()
    return nc


def _pack_windows(D):
    """Greedy 4-D balanced packing of 12500 dst nodes into 98 windows.

    D: [NPC, NG] per-node src-quarter in-degree counts. Returns (win, cls):
    window id and in-window class (0..127) per node. Objective: per-window
    per-quarter edge sums <= 512 (4 slots of 128) wherever possible.
    """
    cap = 4 * P  # 512 edges per (window, quarter) target
    S = np.zeros((NTILE, NG), dtype=np.int64)
    n = np.zeros(NTILE, dtype=np.int64)
    win = np.empty(NPC, dtype=np.int64)
    cls = np.empty(NPC, dtype=np.int64)
    order = np.argsort(-D.max(axis=1), kind="stable")
    for v in order:
        d = D[v]
        over = np.maximum(S + d - cap, 0).sum(axis=1)
        load = (S + d).max(axis=1)
        score = over * 100000 + load + n  # prefer no overflow, then balance
        score[n >= P] = np.iinfo(np.int64).max
        w = int(np.argmin(score))
        win[v] = w
        cls[v] = n[w]
        S[w] += d
        n[w] += 1
    return win, cls


def _preprocess(x, edge_index, W, b):
    """Host-side sharding: layout transforms + integer bucketing/sorting."""
    src = np.asarray(edge_index[0], dtype=np.int64)
    dst = np.asarray(edge_index[1], dtype=np.int64)
    x = np.asarray(x, dtype=np.float32)
    W = np.asarray(W, dtype=np.float32)
    b = np.asarray(b, dtype=np.float32)

    # host layout transpose of x (pure data movement), padded to NALL
    xT = np.zeros((DIN, NALL), dtype=np.float32)
    xT[:, :N_NODES] = x.T

    # out-degree counts, quarter-padded layout: position u = g*QNP + r maps
    # to node g*QN + r (r < QNP; zero beyond N_NODES)
    ocnt_full = np.bincount(src, minlength=N_NODES).astype(np.int32)
    upos = (np.arange(NG * QNP) // QNP) * QN + np.arange(NG * QNP) % QNP
    ocnt_pad = np.where(upos < N_NODES,
                        ocnt_full[np.minimum(upos, N_NODES - 1)], 0)
    ocnt_arr = ocnt_pad.reshape(NCH, P).T.copy().astype(np.int32)

    core_of = dst // NPC
    per_core = []
    all_counts = np.zeros((N_CORES, NTILE, NG), dtype=np.int64)
    for k in range(N_CORES):
        sel = core_of == k
        s_k = src[sel]
        d_k = dst[sel] - k * NPC
        grp = s_k // QN
        # per-node src-quarter in-degree profile, then balanced packing
        D = np.bincount(d_k * NG + grp, minlength=NPC * NG).reshape(NPC, NG)
        win_map, cls_map = _pack_windows(D)
        wv = win_map[d_k]
        cv = cls_map[d_k]
        order = np.lexsort((s_k, grp, wv))
        s_k = s_k[order]
        cv = cv[order]
        wv = wv[order]
        grp = grp[order]
        wg = wv * NG + grp
        wg_counts = np.bincount(wg, minlength=NTILE * NG)
        all_counts[k] = wg_counts.reshape(NTILE, NG)
        node_deg = D.sum(axis=1)  # in-degree per local node
        per_core.append((s_k, cv, wv, grp, wg_counts, win_map, cls_map,
                         node_deg))

    # SPMD-uniform slot structure: mains capped at 4 blocks, excess spills
    # into per-(span, quarter) overflow blocks
    RQW_arr = np.minimum((all_counts.max(axis=0) + P - 1) // P, 4)
    cap_wg = RQW_arr * P  # [NTILE, NG] main capacity
    nspan = (NTILE + SPAN_W - 1) // SPAN_W
    spill = np.maximum(all_counts - cap_wg[None], 0)  # [cores, NTILE, NG]
    spill_sp = np.zeros((N_CORES, nspan, NG), dtype=np.int64)
    for sp in range(nspan):
        spill_sp[:, sp] = spill[:, sp * SPAN_W:(sp + 1) * SPAN_W].sum(axis=1)
    OVN_arr = (spill_sp.max(axis=0) + P - 1) // P  # [nspan, NG]

    RQW = tuple(tuple(int(v) for v in row) for row in RQW_arr)
    OVN = tuple(tuple(int(v) for v in row) for row in OVN_arr)
    lay = _stream_layout(RQW, OVN)
    GSL = lay["GSL"]
    woff = np.asarray(lay["woff"])
    SLOTS = lay["SLOTS"]
    moff_arr = np.zeros((NG, NTILE), dtype=np.int64)
    for g in range(NG):
        for w in range(NTILE):
            moff_arr[g, w] = lay["moff"][g][w]
    # slot offset of (w, g) within the window's slot block (g-major)
    gwoff = np.zeros((NTILE, NG), dtype=np.int64)
    np.cumsum(RQW_arr[:, :-1], axis=1, out=gwoff[:, 1:])

    OVW = SPAN_W * P
    iota_rep = np.broadcast_to(
        np.arange(OVW, dtype=np.float16)[None, :], (P, OVW)
    ).copy()
    b_rep = np.broadcast_to(b[None, :], (P, DOUT)).copy()

    in_maps = []
    inv_perms = []
    for k in range(N_CORES):
        (s_k, cv, wv, grp, wg_counts, win_map, cls_map,
         node_deg) = per_core[k]
        # pad slots must point at a real edge source (outdeg >= 1): with the
        # device's max() removed, zero-degree rows hold inf and 0*inf = NaN
        e_src = []
        for g in range(NG):
            mg = grp == g
            pad_local = int(s_k[mg][0] - g * QN) if mg.any() else 0
            e_src.append(np.full(int(GSL[g]) * P, pad_local, dtype=np.int64))
        e_dst = np.full(SLOTS * P, P, dtype=np.float16)  # in-window class

        wg_starts = np.concatenate([[0], np.cumsum(wg_counts)])[:-1]
        n_e = len(s_k)
        pos_in_run = np.arange(n_e) - np.repeat(wg_starts, wg_counts)
        is_main = pos_in_run < cap_wg[wv, grp]

        # mains: per-quarter stream position moff[g][w]*128 + pos;
        # one-hot position (woff[w] + gwoff[w,g])*128 + pos
        jg = moff_arr[grp, wv] * P + pos_in_run
        js = (woff[wv] + gwoff[wv, grp]) * P + pos_in_run
        for g in range(NG):
            m = (grp == g) & is_main
            e_src[g][jg[m]] = s_k[m] - g * QN
        e_dst[js[is_main]] = cv[is_main].astype(np.float16)

        # overflow: per (span, quarter), w-major order
        spv = wv // SPAN_W
        for sp in range(nspan):
            for g in range(NG):
                if OVN_arr[sp, g] == 0:
                    continue
                m = (~is_main) & (spv == sp) & (grp == g)
                cnt = int(m.sum())
                rank = np.arange(cnt)
                ov_stream0 = (lay["soff"][g][sp + 1] - int(OVN_arr[sp, g]))
                e_src[g][ov_stream0 * P + rank] = s_k[m] - g * QN
                ov_dst0 = lay["ovoff"][g][sp]
                pad_dst = np.full(int(OVN_arr[sp, g]) * P, 1000.0,
                                  dtype=np.float16)
                pad_dst[rank] = (cv[m] + P * (wv[m] - sp * SPAN_W)
                                 ).astype(np.float16)
                e_dst[ov_dst0 * P:(ov_dst0 + int(OVN_arr[sp, g])) * P] = \
                    pad_dst

        qidx_arrs = {}
        for g in range(NG):
            flat = e_src[g].astype(np.int16)
            qi = flat.reshape(int(GSL[g]) * P // 16, 16).T
            qidx_arrs[f"qidx{g}"] = np.tile(qi, (8, 1))
        dstloc_arr = e_dst.reshape(SLOTS, P).T.copy()

        # in-degree counts by packed position l = w*128 + cls
        icnt = np.zeros(NPAD, dtype=np.int32)
        icnt[win_map * P + cls_map] = node_deg.astype(np.int32)
        in_maps.append({
            "xT": xT, "w_in": W, "b_rep": b_rep,
            **qidx_arrs,
            "dstloc": dstloc_arr, "iota_in": iota_rep,
            "ocnt": ocnt_arr, "icnt": icnt.reshape(NTILE, P).T.copy(),
        })
        inv_perms.append(win_map * P + cls_map)

    return in_maps, (RQW, OVN), inv_perms


_prog_cache = {}
_last_results = None


def kernel(x, edge_index, W, b):
    from concourse import bass_utils

    in_maps, key, inv_perms = _preprocess(x, edge_index, W, b)
    if key not in _prog_cache:
        _prog_cache[key] = _build_program(*key)
    nc = _prog_cache[key]

    res = bass_utils.run_bass_kernel_spmd(
        nc, in_maps, core_ids=list(range(N_CORES))
    )
    global _last_results
    _last_results = res
    outs = []
    for k in range(N_CORES):
        o = res.results[k]["out_d"]  # [NPAD, DOUT], packed pos = w*128 + cls
        outs.append(o[inv_perms[k]])
    return np.concatenate(outs, axis=0).astype(np.float32)
